# revision 9
# baseline (speedup 1.0000x reference)
"""Bass/Tile kernel for nn_CorticalRegion (topk_masking) on 8 TRN2 NeuronCores.

Sharding: column dim C=256 split 32-per-core.  Inside each core, columns are
processed in pairs: partition axis = (col-in-pair * 64 + batch), free axis = N.

Per column pair:
  drive  = x @ W_ff + 0.3*x_ctx @ W_ctx + bias            (PE, fp32, psum)
  relud  = relu(drive)                                    (ACT, psum->sbuf)
  boosted= drive + log1p(0.05/(avg+1e-6))                 (PE k=2 bf16 hi/lo rows)
  top-24 threshold via vector max8 / match_replace (3 rounds of 8)
  act    = (boosted >= t) * relud, row sums fused         (DVE scalar_tensor_tensor)
  act_out= act * K/(sum+1e-8)                             (DVE tensor_scalar)
  act_T  = transpose(act_out)                             (PE transpose via identity)
  pred   = act_T.T @ W_pred                               (PE)
  err    = x - pred                                       (DVE)

Host side only does input marshalling (shard slicing, tiny transposes of x,
bf16 hi/lo split of bias/boost rows).
"""

import numpy as np
import ml_dtypes
from contextlib import ExitStack

import concourse.bass as bass  # noqa: F401
import concourse.mybir as mybir
import concourse.tile as tile
from concourse import bacc
from concourse.bass_utils import run_bass_kernel_spmd
from concourse.masks import make_identity

AF = mybir.ActivationFunctionType
ALU = mybir.AluOpType
F32 = mybir.dt.float32
F32R = mybir.dt.float32r
BF16 = mybir.dt.bfloat16
BF = ml_dtypes.bfloat16

B, D, DCTX, N = 64, 256, 256, 512
C_TOT, K, NCORES = 256, 24, 8
C_LOC = C_TOT // NCORES          # 32 columns per core
NPAIR = C_LOC // 2               # 16 column pairs
FEEDBACK = 0.3
NEG = -1.0e30

# matmul dtype knobs: F32 (exact, 4 cyc/row) or F32R (1 cyc/row @ N>=256)
DRIVE_DT = F32
PRED_DT = F32


def build_nc(drive_dt=None, pred_dt=None):
    drive_dt = DRIVE_DT if drive_dt is None else drive_dt
    pred_dt = PRED_DT if pred_dt is None else pred_dt
    nc = bacc.Bacc("TRN2", target_bir_lowering=False, debug=False)

    xT_d = nc.dram_tensor("xT", (128, 2, B), drive_dt, kind="ExternalInput").ap()
    xcT_d = nc.dram_tensor("xcT", (128, 2, B), drive_dt, kind="ExternalInput").ap()
    xrep_d = nc.dram_tensor("x_rep", (128, D), F32, kind="ExternalInput").ap()
    wff_d = nc.dram_tensor("W_ff", (C_LOC, D, N), drive_dt, kind="ExternalInput").ap()
    wctx_d = nc.dram_tensor("W_ctx", (C_LOC, DCTX, N), drive_dt, kind="ExternalInput").ap()
    wpred_d = nc.dram_tensor("W_pred", (C_LOC, N, D), pred_dt, kind="ExternalInput").ap()
    bbias_d = nc.dram_tensor("bb_bias", (4, 2, C_LOC // 4, N), BF16, kind="ExternalInput").ap()
    bboost_d = nc.dram_tensor("bb_boost", (4, 2, C_LOC // 4, N), BF16, kind="ExternalInput").ap()
    act_d = nc.dram_tensor("activations", (B, C_LOC, N), F32, kind="ExternalOutput").ap()
    pred_d = nc.dram_tensor("predictions", (B, C_LOC, D), F32, kind="ExternalOutput").ap()
    err_d = nc.dram_tensor("errors", (B, C_LOC, D), F32, kind="ExternalOutput").ap()

    with tile.TileContext(nc) as tc, ExitStack() as ctx:
        const = ctx.enter_context(tc.tile_pool(name="const", bufs=1))
        wpool = ctx.enter_context(tc.tile_pool(name="wts", bufs=3))
        work = ctx.enter_context(tc.tile_pool(name="work", bufs=3))
        small = ctx.enter_context(tc.tile_pool(name="small", bufs=4))
        outp = ctx.enter_context(tc.tile_pool(name="outp", bufs=3))
        psA = ctx.enter_context(tc.tile_pool(name="psA", bufs=2, space="PSUM"))
        psB = ctx.enter_context(tc.tile_pool(name="psB", bufs=2, space="PSUM"))
        psC = ctx.enter_context(tc.tile_pool(name="psC", bufs=2, space="PSUM"))
        psD = ctx.enter_context(tc.tile_pool(name="psD", bufs=2, space="PSUM"))

        ident = const.tile([128, 128], F32, tag="ident")
        make_identity(nc, ident)
        ones_bf = const.tile([128, N], BF16, tag="ones")
        nc.vector.memset(ones_bf, 1.0)
        # zero stationary row: matmul(zrow, anything) contributes 0 — used to
        # open each psum bank's accumulation group across all 128 partitions
        zrow = const.tile([128, 128], BF16, tag="zrow")
        nc.vector.memset(zrow, 0.0)
        xT = const.tile([128, 2, B], drive_dt, tag="xT")
        nc.sync.dma_start(xT, xT_d)
        xcT = const.tile([128, 2, B], drive_dt, tag="xcT")
        nc.sync.dma_start(xcT, xcT_d)
        x_rep = const.tile([128, D], F32, tag="xrep")
        nc.sync.dma_start(x_rep, xrep_d)
        bbias = const.tile([128, C_LOC // 4, N], BF16, tag="bbias")
        bboost = const.tile([128, C_LOC // 4, N], BF16, tag="bboost")
        for strip in range(4):
            st = strip * 32
            nc.sync.dma_start(bbias[st:st + 2, :, :], bbias_d[strip])
            nc.sync.dma_start(bboost[st:st + 2, :, :], bboost_d[strip])

        def open_bank(pbank, width):
            nc.tensor.matmul(pbank, zrow[0:1, :], ones_bf[0:1, :width],
                             start=True, stop=False, tile_position=(0, 0))

        def close_bank(pbank, width):
            nc.tensor.matmul(pbank, zrow[0:1, :], ones_bf[0:1, :width],
                             start=False, stop=True, tile_position=(0, 0))

        for j in range(NPAIR):
            c0 = 2 * j
            wff = wpool.tile([128, 2, 2, N], drive_dt, tag="wff")
            wctx = wpool.tile([128, 2, 2, N], drive_dt, tag="wctx")
            wpred = wpool.tile([128, 2, 4, D], pred_dt, tag="wpred")
            for cc in range(2):
                c = c0 + cc
                nc.sync.dma_start(wff[:, cc], wff_d[c].rearrange("(k p) n -> p k n", p=128))
                nc.sync.dma_start(wctx[:, cc], wctx_d[c].rearrange("(k p) n -> p k n", p=128))
                nc.sync.dma_start(wpred[:, cc], wpred_d[c].rearrange("(k p) d -> p k d", p=128))

            # ---- drive matmuls (psum partitions cc*64..cc*64+63 = column c) ----
            # one accumulation group per bank: a zero-contribution matmul opens
            # it across all 128 partitions, everything else accumulates
            ps = psA.tile([128, N], F32, tag="drive")
            open_bank(ps, N)
            for cc in range(2):
                c = c0 + cc
                ob = cc * 64
                out = ps[ob:ob + 64, :]
                tp = (0, ob)
                nc.tensor.matmul(out, xT[:, 0], wff[:, cc, 0], start=False, stop=False, tile_position=tp)
                nc.tensor.matmul(out, xT[:, 1], wff[:, cc, 1], start=False, stop=False, tile_position=tp)
                nc.tensor.matmul(out, xcT[:, 0], wctx[:, cc, 0], start=False, stop=False, tile_position=tp)
                nc.tensor.matmul(out, xcT[:, 1], wctx[:, cc, 1], start=False, stop=False, tile_position=tp)
                st = (c % 4) * 32
                nc.tensor.matmul(out, ones_bf[st:st + 2, :B], bbias[st:st + 2, c // 4, :],
                                 start=False, stop=False, tile_position=(st, ob))
            close_bank(ps, N)
            # homeostatic boost rows broadcast into their own psum bank
            bq = psD.tile([128, N], F32, tag="boostq")
            open_bank(bq, N)
            for cc in range(2):
                c = c0 + cc
                ob = cc * 64
                st = (c % 4) * 32
                nc.tensor.matmul(bq[ob:ob + 64, :], ones_bf[st:st + 2, :B], bboost[st:st + 2, c // 4, :],
                                 start=False, stop=False, tile_position=(st, ob))
            close_bank(bq, N)
            relud = work.tile([128, N], F32, tag="relud")
            nc.scalar.activation(relud, ps, AF.Relu)
            boost_sb = work.tile([128, N], F32, tag="boostsb")
            nc.scalar.copy(boost_sb, bq)
            boosted = work.tile([128, N], F32, tag="boosted")
            nc.vector.tensor_tensor(boosted, ps, boost_sb, ALU.add)

            # ---- top-24 threshold: 3 rounds of max8 (+2 match_replace) ----
            scratch = work.tile([128, N], F32, tag="scratch")
            nc.vector.tensor_copy(scratch, boosted)
            m1 = small.tile([128, 8], F32, tag="m1")
            m2 = small.tile([128, 8], F32, tag="m2")
            m3 = small.tile([128, 8], F32, tag="m3")
            nc.vector.max(m1, scratch)
            nc.vector.match_replace(scratch, m1, scratch, NEG)
            nc.vector.max(m2, scratch)
            nc.vector.match_replace(scratch, m2, scratch, NEG)
            nc.vector.max(m3, scratch)

            # ---- mask + multiply + fused row-sum ----
            act = work.tile([128, N], F32, tag="act")
            ssum = small.tile([128, 1], F32, tag="ssum")
            nc.vector.scalar_tensor_tensor(act, boosted, m3[:, 7:8], relud,
                                           ALU.is_ge, ALU.mult, accum_out=ssum)
            sinv = small.tile([128, 1], F32, tag="sinv")
            nc.vector.tensor_scalar_add(sinv, ssum, 1e-8)
            srec = small.tile([128, 1], F32, tag="srec")
            nc.vector.reciprocal(srec, sinv)
            act_out = work.tile([128, N], F32, tag="actout")
            nc.vector.tensor_scalar(act_out, act, srec, float(K), ALU.mult, ALU.mult)
            nc.sync.dma_start(act_d[:, c0, :], act_out[0:64, :])
            nc.sync.dma_start(act_d[:, c0 + 1, :], act_out[64:128, :])

            # ---- transpose activations for the prediction matmul ----
            pt = psC.tile([128, 4, 128], F32, tag="ptr")
            for q in range(4):
                nc.tensor.transpose(pt[:, q, :], act_out[:, 128 * q:128 * (q + 1)], ident)
            actT = work.tile([128, 4, 128], pred_dt, tag="actT")
            nc.scalar.copy(actT, pt)

            # ---- prediction matmuls + errors ----
            pp = psB.tile([128, D], F32, tag="pred")
            open_bank(pp, D)
            for cc in range(2):
                ob = cc * 64
                for q in range(4):
                    nc.tensor.matmul(pp[ob:ob + 64, :], actT[:, q, ob:ob + 64], wpred[:, cc, q, :],
                                     start=False, stop=False, tile_position=(0, ob))
            close_bank(pp, D)
            err = outp.tile([128, D], F32, tag="err")
            nc.vector.tensor_tensor(err, x_rep, pp, ALU.subtract)
            predsb = outp.tile([128, D], F32, tag="predsb")
            nc.scalar.copy(predsb, pp)
            nc.sync.dma_start(err_d[:, c0, :], err[0:64, :])
            nc.sync.dma_start(err_d[:, c0 + 1, :], err[64:128, :])
            nc.sync.dma_start(pred_d[:, c0, :], predsb[0:64, :])
            nc.sync.dma_start(pred_d[:, c0 + 1, :], predsb[64:128, :])

    nc.compile()
    return nc


def _split_hi_lo_strips(v):
    """[32, N] f32 -> [4, 2, 8, N] bf16 laid out [strip, hi/lo, slot, n],
    where column c sits at (strip=c%4, slot=c//4)."""
    hi = v.astype(BF)
    lo = (v - hi.astype(np.float32)).astype(BF)
    arr = np.stack([hi, lo], axis=0)            # [2, 32, N]
    arr = arr.reshape(2, C_LOC // 4, 4, N).transpose(2, 0, 1, 3)
    return np.ascontiguousarray(arr)


def prep_in_maps(x_input, x_context, W_ff, W_ctx, W_pred, bias, avg_activity):
    x_input = np.asarray(x_input, np.float32)
    x_context = np.asarray(x_context, np.float32)
    xT = np.ascontiguousarray(x_input.T.reshape(2, 128, B).transpose(1, 0, 2))
    xcT = np.ascontiguousarray(
        (np.float32(FEEDBACK) * x_context).T.reshape(2, 128, B).transpose(1, 0, 2))
    x_rep = np.ascontiguousarray(np.concatenate([x_input, x_input], axis=0))
    W_ff = np.asarray(W_ff, np.float32)
    W_ctx = np.asarray(W_ctx, np.float32)
    W_pred = np.asarray(W_pred, np.float32)
    bias = np.asarray(bias, np.float32)
    avg = np.asarray(avg_activity, np.float32)
    boost = np.log1p(np.float32(0.05) / (avg + np.float32(1e-6))).astype(np.float32)

    in_maps = []
    for g in range(NCORES):
        sl = slice(g * C_LOC, (g + 1) * C_LOC)
        in_maps.append({
            "xT": xT,
            "xcT": xcT,
            "x_rep": x_rep,
            "W_ff": np.ascontiguousarray(W_ff[sl]),
            "W_ctx": np.ascontiguousarray(W_ctx[sl]),
            "W_pred": np.ascontiguousarray(W_pred[sl]),
            "bb_bias": _split_hi_lo_strips(bias[sl]),
            "bb_boost": _split_hi_lo_strips(boost[sl]),
        })
    return in_maps


_NC_CACHE = {}


def get_nc(drive_dt=None, pred_dt=None):
    key = (drive_dt or DRIVE_DT, pred_dt or PRED_DT)
    if key not in _NC_CACHE:
        _NC_CACHE[key] = build_nc(*key)
    return _NC_CACHE[key]


def _assemble(results):
    acts = np.concatenate([r["activations"] for r in results], axis=1)
    preds = np.concatenate([r["predictions"] for r in results], axis=1)
    errs = np.concatenate([r["errors"] for r in results], axis=1)
    return acts, preds, errs


def kernel(x_input, x_context, W_ff, W_ctx, W_pred, bias, avg_activity):
    nc = get_nc()
    in_maps = prep_in_maps(x_input, x_context, W_ff, W_ctx, W_pred, bias, avg_activity)
    res = run_bass_kernel_spmd(nc, in_maps, core_ids=list(range(NCORES)))
    return _assemble(res.results)
